# revision 5
# baseline (speedup 1.0000x reference)
"""fp8 quantized matmul y = fp8(x) @ fp8(W)^T on 8 Trainium2 NeuronCores.

Reference semantics: x[M,K] f32 and W[N,K] f32 are each cast to
float8_e4m3fn (OCP, round-to-nearest-even) and the matmul accumulates in
fp32.  The cast is a pure element-wise dtype conversion, done here on the
host with ml_dtypes (bit-identical to the reference's jax cast for the
value range involved: |x| < 16, |W| <= 2^-6, both far below 240 where the
OCP and IEEE e4m3 encodings coincide).

Sharding: data-parallel over M — each of the 8 cores computes a
[1024, 4096] slice of y from its x shard; W^T (16 MiB as fp8) is
replicated and lives entirely in SBUF.

Device kernel: fp8 DoubleRow matmuls (2 contraction rows per PE cell).
DoubleRow's LDWEIGHTS streams 256 stationary columns (~183 ns) — longer
than the 512-wide fp8 moving stream (~120 ns) — so a naive
one-LDW-per-matmul schedule is weight-load bound.  Here each stationary
x-tile slice is reused across G=4 consecutive matmuls that stream 4
different W n-tiles into 4 PSUM banks (the redundant LDWEIGHTS are elided
downstream / pipelined 4-deep), so the PE runs at the moving-stream rate.
The 4-bank waves are double-buffered across the 8 PSUM banks; DVE
evacuates PSUM -> SBUF as fp16 (halving store traffic; quantization noise
~5e-4 relative, far under the 2e-2 gate) and the stores ride the ACT
HWDGE ring so they never queue behind input loads on the SP ring.

Host-side layouts are pre-transposed so every DMA is a large contiguous
per-partition transfer:
  xt[mt, p, kt, m] = fp8(x_shard)[mt*128 + m, kt*128 + p]   (4 MiB/core)
  wt[nt, p, kt, n] = fp8(W)[nt*512 + n, kt*128 + p]         (16 MiB)
The [p, kt, cols] SBUF tiles feed nc.tensor.matmul sliced
[:, 2t:2t+2, :] — the DoubleRow contraction pair is (kt*128+p) over two
consecutive kt subtiles, identically on both operands.
"""

import numpy as np
import ml_dtypes

P = 128          # partitions
N_CORES = 8
M, K, N = 8192, 4096, 4096
MC = M // N_CORES          # 1024 rows of x per core
MT = MC // P               # 8 m-tiles per core
KT = K // P                # 32 k-subtiles
NB = 512                   # psum bank width (f32)
NT = N // NB               # 8 n-tiles

_NC_CACHE = {}


N_WARMUP = 8  # dummy PE matmuls bridging the startup barrier -> first data


def _emit(nc, tc, mybir, X, W, Y, mt_n, nt_n, kt_n, nb):
    fp8 = mybir.dt.float8e4
    f32 = mybir.dt.float32
    f16 = mybir.dt.float16
    import contextlib

    kq = min(8, kt_n)          # kt-subtiles per W DMA group
    wg_n = kt_n // kq          # W DMA groups per nt column
    assert kt_n % kq == 0 and kq % 2 == 0
    G = min(4, nt_n)           # n-tiles sharing one stationary load
    q_n = nt_n // G            # stationary-reuse waves over nt
    assert nt_n % G == 0
    n_pairs = kt_n // 2
    pairs_per_g = kq // 2

    with contextlib.ExitStack() as ctx:
        warm = ctx.enter_context(tc.tile_pool(name="warm", bufs=1))
        xpool = ctx.enter_context(tc.tile_pool(name="xpool", bufs=1))
        wpool = ctx.enter_context(tc.tile_pool(name="wpool", bufs=1))
        spool = ctx.enter_context(tc.tile_pool(name="spool", bufs=8))
        ppool = ctx.enter_context(
            tc.tile_pool(name="ppool", bufs=2, space="PSUM")
        )

        # PE warmup on memset tiles: occupies the tensor engine from the
        # end of the startup barrier until the first input DMAs land, so
        # the HAM clock gate is released before real matmuls begin.
        wm_x = warm.tile([P, 2, P], fp8, name="wm_x", tag="wm_x")
        wm_w = warm.tile([P, 2, nb], fp8, name="wm_w", tag="wm_w")
        nc.gpsimd.memset(wm_x, 0.0)
        nc.gpsimd.memset(wm_w, 0.0)
        # shares PSUM tag ps0 with the main waves; the first real wave's
        # allocation just waits for the warmup matmuls to retire
        wm_ps = ppool.tile([P, nb], f32, name="wm_ps", tag="ps0")
        for _ in range(N_WARMUP):
            nc.tensor.matmul(
                wm_ps,
                wm_x,
                wm_w,
                start=True,
                stop=True,
                perf_mode=mybir.MatmulPerfMode.DoubleRow,
            )

        # Input loads, all on the SP HWDGE ring, in consumption order:
        # x0, then wave-0 W groups g-interleaved with x1..x3 so the first
        # mt-wave's later kt-groups and the next waves' x tiles both land
        # on time, then the remaining x and W quads.
        xt = [None] * mt_n
        wt = [[None] * wg_n for _ in range(nt_n)]
        # wave-0's first kt-group arrives as per-pair 128 KiB slivers so the
        # first matmul only waits on ~1 MiB of DMA instead of 2.5 MiB
        wfine = [[None] * pairs_per_g for _ in range(G)]

        def load_x(mt):
            t = xpool.tile([P, kt_n, P], fp8, name=f"xt{mt}", tag=f"xt{mt}")
            nc.sync.dma_start(out=t, in_=X[mt, :, :, :])
            xt[mt] = t

        def load_w(nt, g):
            t = wpool.tile([P, kq, nb], fp8, name=f"wt{nt}_{g}", tag=f"wt{nt}_{g}")
            nc.sync.dma_start(out=t, in_=W[nt, :, g * kq : (g + 1) * kq, :])
            wt[nt][g] = t

        def load_w_fine(j, s):
            t = wpool.tile([P, 2, nb], fp8, name=f"wf{j}_{s}", tag=f"wf{j}_{s}")
            nc.sync.dma_start(out=t, in_=W[j, :, 2 * s : 2 * s + 2, :])
            wfine[j][s] = t

        load_x(0)
        for s in range(pairs_per_g):
            for j in range(G):
                load_w_fine(j, s)
        for g in range(1, wg_n):
            for j in range(G):
                load_w(j, g)
            if g < mt_n:
                load_x(g)
        for mt in range(wg_n, mt_n):
            load_x(mt)
        for q in range(1, q_n):
            for g in range(wg_n):
                for j in range(G):
                    load_w(q * G + j, g)

        def w_slice(q, j, t2):
            g, lp = divmod(t2, pairs_per_g)
            if q == 0 and g == 0 and j < G:
                return wfine[j][lp][:, 0:2, :]
            return wt[q * G + j][g][:, 2 * lp : 2 * lp + 2, :]

        def store(q, mt, j, ps_t, n_off, n_len):
            st = spool.tile([P, n_len], f16, name="st", tag=f"st{j}")
            nc.vector.tensor_copy(out=st, in_=ps_t[:, n_off : n_off + n_len])
            # outputs ride the ACT HWDGE ring so they never queue
            # behind the input loads on the SP ring
            nt = q * G + j
            nc.scalar.dma_start(
                out=Y[
                    mt * P : (mt + 1) * P,
                    nt * nb + n_off : nt * nb + n_off + n_len,
                ],
                in_=st,
            )

        def emit_wave(q, mt):
            ps = [
                ppool.tile([P, nb], f32, name=f"ps{j}", tag=f"ps{j}")
                for j in range(G)
            ]
            for t2 in range(n_pairs):
                xs = xt[mt][:, 2 * t2 : 2 * t2 + 2, :]
                for j in range(G):
                    nc.tensor.matmul(
                        ps[j],
                        xs,
                        w_slice(q, j, t2),
                        start=(t2 == 0),
                        stop=(t2 == n_pairs - 1),
                        perf_mode=mybir.MatmulPerfMode.DoubleRow,
                    )
            for j in range(G):
                store(q, mt, j, ps[j], 0, nb)

        def emit_last_wave(q, mt):
            # serialize the final wave bank-by-bank so each bank's PSUM
            # eviction + store overlaps the next bank's matmuls; the very
            # last bank is evicted in halves to pipeline DVE with the DMA
            for j in range(G):
                ps_t = ppool.tile([P, nb], f32, name=f"ps{j}", tag=f"ps{j}")
                for t2 in range(n_pairs):
                    nc.tensor.matmul(
                        ps_t,
                        xt[mt][:, 2 * t2 : 2 * t2 + 2, :],
                        w_slice(q, j, t2),
                        start=(t2 == 0),
                        stop=(t2 == n_pairs - 1),
                        perf_mode=mybir.MatmulPerfMode.DoubleRow,
                    )
                if j == G - 1:
                    store(q, mt, j, ps_t, 0, nb // 2)
                    store(q, mt, j, ps_t, nb // 2, nb - nb // 2)
                else:
                    store(q, mt, j, ps_t, 0, nb)

        for q in range(q_n):
            for mt in range(mt_n):
                if q == q_n - 1 and mt == mt_n - 1:
                    emit_last_wave(q, mt)
                else:
                    emit_wave(q, mt)


def _build(mt_n=MT, nt_n=NT, kt_n=KT, nb=NB, hw=True):
    import concourse.bacc as bacc
    import concourse.mybir as mybir
    import concourse.tile as tile
    from concourse.bass_interp import get_hw_module

    nc = bacc.Bacc("TRN2", target_bir_lowering=False, debug=False)
    X = nc.dram_tensor(
        "xt", [mt_n, P, kt_n, P], mybir.dt.float8e4, kind="ExternalInput"
    ).ap()
    W = nc.dram_tensor(
        "wt", [nt_n, P, kt_n, nb], mybir.dt.float8e4, kind="ExternalInput"
    ).ap()
    Y = nc.dram_tensor(
        "y", [mt_n * P, nt_n * nb], mybir.dt.float16, kind="ExternalOutput"
    ).ap()
    with tile.TileContext(nc) as tc:
        _emit(nc, tc, mybir, X, W, Y, mt_n, nt_n, kt_n, nb)
    nc.compile()
    if hw:
        nc.m = get_hw_module(nc.m)
    return nc


def _get_nc():
    if "nc" not in _NC_CACHE:
        _NC_CACHE["nc"] = _build()
    return _NC_CACHE["nc"]


def _quantize(a):
    # OCP e4m3fn RNE cast (matches jax astype), then reinterpret as the
    # IEEE e4m3 dtype the BIR tensor declares (identical bits below 240).
    return a.astype(ml_dtypes.float8_e4m3fn).view(ml_dtypes.float8_e4m3)


def _in_maps(x, W):
    xq = _quantize(np.ascontiguousarray(x))
    wq = _quantize(np.ascontiguousarray(W))
    # wt[nt, p, kt, n] = wq[nt*NB + n, kt*P + p]
    wt = np.ascontiguousarray(wq.reshape(NT, NB, KT, P).transpose(0, 3, 2, 1))
    maps = []
    for c in range(N_CORES):
        xc = xq[c * MC : (c + 1) * MC]
        # xt[mt, p, kt, m] = xc[mt*P + m, kt*P + p]
        xt = np.ascontiguousarray(xc.reshape(MT, P, KT, P).transpose(0, 3, 2, 1))
        maps.append({"xt": xt, "wt": wt})
    return maps


def _ensure_axon_ntff_hook():
    # Under axon, run_bass_kernel_spmd(trace=True) imports
    # antenv.axon_hooks, which some images lack even though the boot
    # machinery that implements the hook is present.  Register a shim so
    # tracing degrades gracefully instead of raising.
    import sys

    if "antenv.axon_hooks" in sys.modules:
        return
    try:
        from concourse._compat import axon_active

        if not axon_active():
            return
        import importlib.util

        if importlib.util.find_spec("antenv.axon_hooks") is not None:
            return
        import types

        import antenv

        hook = None
        try:
            import trn_agent_boot.trn_boot as _tb

            hook = _tb._ntff_profile_via_ctypes("/opt/axon/libaxon_pjrt.so")
        except Exception:
            hook = None
        mod = types.ModuleType("antenv.axon_hooks")
        mod._hook = hook
        mod.get_axon_ntff_profile_hook = lambda: mod._hook
        def _set(h):
            mod._hook = h
        mod.set_axon_ntff_profile_hook = _set
        antenv.axon_hooks = mod
        sys.modules["antenv.axon_hooks"] = mod
    except Exception:
        pass


def _run(in_maps, trace=False):
    from concourse.bass_utils import run_bass_kernel_spmd

    _ensure_axon_ntff_hook()
    nc = _get_nc()
    return run_bass_kernel_spmd(
        nc, in_maps, core_ids=list(range(len(in_maps))), trace=trace
    )


def kernel(x, W):
    res = _run(_in_maps(x, W))
    return np.concatenate(
        [res.results[c]["y"] for c in range(N_CORES)], axis=0
    ).astype(np.float32, copy=False)


# revision 10
# speedup vs baseline: 1.0055x; 1.0055x over previous
"""fp8 quantized matmul y = fp8(x) @ fp8(W)^T on 8 Trainium2 NeuronCores.

Reference semantics: x[M,K] f32 and W[N,K] f32 are each cast to
float8_e4m3fn (OCP, round-to-nearest-even) and the matmul accumulates in
fp32.  The cast is a pure element-wise dtype conversion, done here on the
host with ml_dtypes (bit-identical to the reference's jax cast for the
value range involved: |x| < 16, |W| <= 2^-6, both far below 240 where the
OCP and IEEE e4m3 encodings coincide).

Sharding: data-parallel over M — each of the 8 cores computes a
[1024, 4096] slice of y from its x shard; W^T (16 MiB as fp8) is
replicated and lives entirely in SBUF.

Device kernel: fp8 DoubleRow matmuls (2 contraction rows per PE cell).
DoubleRow's LDWEIGHTS streams 256 stationary columns (~183 ns) — longer
than the 512-wide fp8 moving stream (~120 ns) — so a naive
one-LDW-per-matmul schedule is weight-load bound.  Here each stationary
x-tile slice is reused across G=4 consecutive matmuls that stream 4
different W n-tiles into 4 PSUM banks (the redundant LDWEIGHTS are elided
downstream / pipelined 4-deep), so the PE runs at the moving-stream rate.
The 4-bank waves are double-buffered across the 8 PSUM banks; DVE
evacuates PSUM -> SBUF as fp16 (halving store traffic; quantization noise
~5e-4 relative, far under the 2e-2 gate) and the stores ride the ACT
HWDGE ring so they never queue behind input loads on the SP ring.

Host-side layouts are pre-transposed so every DMA is a large contiguous
per-partition transfer:
  xt[mt, p, kt, m] = fp8(x_shard)[mt*128 + m, kt*128 + p]   (4 MiB/core)
  wt[nt, p, kt, n] = fp8(W)[nt*512 + n, kt*128 + p]         (16 MiB)
The [p, kt, cols] SBUF tiles feed nc.tensor.matmul sliced
[:, 2t:2t+2, :] — the DoubleRow contraction pair is (kt*128+p) over two
consecutive kt subtiles, identically on both operands.
"""

import numpy as np
import ml_dtypes

P = 128          # partitions
N_CORES = 8
M, K, N = 8192, 4096, 4096
MC = M // N_CORES          # 1024 rows of x per core
MT = MC // P               # 8 m-tiles per core
KT = K // P                # 32 k-subtiles
NB = 512                   # psum bank width (f32)
NT = N // NB               # 8 n-tiles

_NC_CACHE = {}


N_WARMUP = 10  # dummy PE matmuls bridging the startup barrier -> first data


def _emit(nc, tc, mybir, X, W, Y, mt_n, nt_n, kt_n, nb):
    fp8 = mybir.dt.float8e4
    f32 = mybir.dt.float32
    f16 = mybir.dt.float16
    import contextlib

    kq = min(8, kt_n)          # kt-subtiles per W DMA group
    wg_n = kt_n // kq          # W DMA groups per nt column
    assert kt_n % kq == 0 and kq % 2 == 0
    G = min(4, nt_n)           # n-tiles sharing one stationary load
    q_n = nt_n // G            # stationary-reuse waves over nt
    assert nt_n % G == 0
    n_pairs = kt_n // 2
    pairs_per_g = kq // 2

    with contextlib.ExitStack() as ctx:
        warm = ctx.enter_context(tc.tile_pool(name="warm", bufs=1))
        xpool = ctx.enter_context(tc.tile_pool(name="xpool", bufs=1))
        wpool = ctx.enter_context(tc.tile_pool(name="wpool", bufs=1))
        spool = ctx.enter_context(tc.tile_pool(name="spool", bufs=8))
        ppool = ctx.enter_context(
            tc.tile_pool(name="ppool", bufs=2, space="PSUM")
        )

        # PE warmup on memset tiles: occupies the tensor engine from the
        # end of the startup barrier until the first input DMAs land, so
        # the HAM clock gate is released before real matmuls begin.
        wm_x = warm.tile([P, 2, P], fp8, name="wm_x", tag="wm_x")
        wm_w = warm.tile([P, 2, nb], fp8, name="wm_w", tag="wm_w")
        nc.gpsimd.memset(wm_x, 0.0)
        nc.gpsimd.memset(wm_w, 0.0)
        # shares PSUM tag ps0 with the main waves; the first real wave's
        # allocation just waits for the warmup matmuls to retire
        wm_ps = ppool.tile([P, nb], f32, name="wm_ps", tag="ps0")
        for _ in range(N_WARMUP):
            nc.tensor.matmul(
                wm_ps,
                wm_x,
                wm_w,
                start=True,
                stop=True,
                perf_mode=mybir.MatmulPerfMode.DoubleRow,
            )

        # Input loads, all on the SP HWDGE ring, in consumption order:
        # x0, then wave-0 W groups g-interleaved with x1..x3 so the first
        # mt-wave's later kt-groups and the next waves' x tiles both land
        # on time, then the remaining x and W quads.
        xt = [None] * mt_n
        wt = [[None] * wg_n for _ in range(nt_n)]
        # the first two m-tiles and wave-0's first kt-group arrive as fine
        # slivers so the interleaved opening wave can start on ~0.75 MiB of
        # DMA and its per-kt-group appetite (~290 GB/s with two m-tiles
        # interleaved) stays below the ~378 GB/s the SP ring delivers
        xfine = [[None] * wg_n for _ in range(2)]
        wfine = [[None] * pairs_per_g for _ in range(G)]

        def load_x(mt):
            t = xpool.tile([P, kt_n, P], fp8, name=f"xt{mt}", tag=f"xt{mt}")
            nc.sync.dma_start(out=t, in_=X[mt, :, :, :])
            xt[mt] = t

        def load_x_fine(mt, s):
            t = xpool.tile([P, kq, P], fp8, name=f"xf{mt}_{s}", tag=f"xf{mt}_{s}")
            nc.sync.dma_start(out=t, in_=X[mt, :, s * kq : (s + 1) * kq, :])
            xfine[mt][s] = t

        def load_w(nt, g):
            t = wpool.tile([P, kq, nb], fp8, name=f"wt{nt}_{g}", tag=f"wt{nt}_{g}")
            nc.sync.dma_start(out=t, in_=W[nt, :, g * kq : (g + 1) * kq, :])
            wt[nt][g] = t

        def load_w_fine(j, s):
            t = wpool.tile([P, 2, nb], fp8, name=f"wf{j}_{s}", tag=f"wf{j}_{s}")
            nc.sync.dma_start(out=t, in_=W[j, :, 2 * s : 2 * s + 2, :])
            wfine[j][s] = t

        # consumption-ordered: the interleaved waves 0+1 eat x0/x1 kt-group
        # s and W kt-pair k in lockstep
        load_x_fine(0, 0)
        load_x_fine(1, 0)
        for s in range(pairs_per_g):
            for j in range(G):
                load_w_fine(j, s)
        for g in range(1, wg_n):
            load_x_fine(0, g)
            load_x_fine(1, g)
            for j in range(G):
                load_w(j, g)
        for mt in range(2, mt_n):
            load_x(mt)
        for q in range(1, q_n):
            for g in range(wg_n):
                for j in range(G):
                    load_w(q * G + j, g)

        def x_slice(mt, t2):
            if mt < 2:
                g, lp = divmod(t2, pairs_per_g)
                return xfine[mt][g][:, 2 * lp : 2 * lp + 2, :]
            return xt[mt][:, 2 * t2 : 2 * t2 + 2, :]

        def w_slice(q, j, t2):
            g, lp = divmod(t2, pairs_per_g)
            if q == 0 and g == 0 and j < G:
                return wfine[j][lp][:, 0:2, :]
            return wt[q * G + j][g][:, 2 * lp : 2 * lp + 2, :]

        def store(q, mt, j, ps_t, n_off, n_len):
            st = spool.tile([P, n_len], f16, name="st", tag=f"st{j}")
            nc.vector.tensor_copy(out=st, in_=ps_t[:, n_off : n_off + n_len])
            # outputs ride the ACT HWDGE ring so they never queue
            # behind the input loads on the SP ring
            nt = q * G + j
            nc.scalar.dma_start(
                out=Y[
                    mt * P : (mt + 1) * P,
                    nt * nb + n_off : nt * nb + n_off + n_len,
                ],
                in_=st,
            )

        def emit_wave01():
            # waves mt=0 and mt=1 interleaved across all 8 PSUM banks so
            # the opening fresh-W appetite is halved (DMA keeps up and the
            # PE never idles into a HAM re-throttle); the last 3 kt-pairs
            # run staggered so wave-0's eviction hides under wave-1's tail
            stag = 3
            ps = [
                [
                    ppool.tile([P, nb], f32, name=f"ps{w}_{j}", tag=f"ps{j}")
                    for j in range(G)
                ]
                for w in range(2)
            ]

            def mms(w, t2):
                for j in range(G):
                    nc.tensor.matmul(
                        ps[w][j],
                        x_slice(w, t2),
                        w_slice(0, j, t2),
                        start=(t2 == 0),
                        stop=(t2 == n_pairs - 1),
                        perf_mode=mybir.MatmulPerfMode.DoubleRow,
                    )

            for t2 in range(n_pairs - stag):
                for w in range(2):
                    mms(w, t2)
            for t2 in range(n_pairs - stag, n_pairs):
                mms(0, t2)
            for j in range(G):
                store(0, 0, j, ps[0][j], 0, nb)
            for t2 in range(n_pairs - stag, n_pairs):
                mms(1, t2)
            for j in range(G):
                store(0, 1, j, ps[1][j], 0, nb)

        def emit_wave(q, mt):
            ps = [
                ppool.tile([P, nb], f32, name=f"ps{j}", tag=f"ps{j}")
                for j in range(G)
            ]
            for t2 in range(n_pairs):
                xs = x_slice(mt, t2)
                for j in range(G):
                    nc.tensor.matmul(
                        ps[j],
                        xs,
                        w_slice(q, j, t2),
                        start=(t2 == 0),
                        stop=(t2 == n_pairs - 1),
                        perf_mode=mybir.MatmulPerfMode.DoubleRow,
                    )
            for j in range(G):
                store(q, mt, j, ps[j], 0, nb)

        def emit_last_wave(q, mt):
            # serialize the final wave bank-by-bank so each bank's PSUM
            # eviction + store overlaps the next bank's matmuls; the very
            # last bank is evicted in halves to pipeline DVE with the DMA
            for j in range(G):
                ps_t = ppool.tile([P, nb], f32, name=f"ps{j}", tag=f"ps{j}")
                for t2 in range(n_pairs):
                    nc.tensor.matmul(
                        ps_t,
                        x_slice(mt, t2),
                        w_slice(q, j, t2),
                        start=(t2 == 0),
                        stop=(t2 == n_pairs - 1),
                        perf_mode=mybir.MatmulPerfMode.DoubleRow,
                    )
                if j == G - 1:
                    store(q, mt, j, ps_t, 0, nb // 2)
                    store(q, mt, j, ps_t, nb // 2, nb - nb // 2)
                else:
                    store(q, mt, j, ps_t, 0, nb)

        emit_wave01()
        for q in range(q_n):
            for mt in range(2 if q == 0 else 0, mt_n):
                if q == q_n - 1 and mt == mt_n - 1:
                    emit_last_wave(q, mt)
                else:
                    emit_wave(q, mt)


def _build(mt_n=MT, nt_n=NT, kt_n=KT, nb=NB, hw=True):
    import concourse.bacc as bacc
    import concourse.mybir as mybir
    import concourse.tile as tile
    from concourse.bass_interp import get_hw_module

    nc = bacc.Bacc("TRN2", target_bir_lowering=False, debug=False)
    X = nc.dram_tensor(
        "xt", [mt_n, P, kt_n, P], mybir.dt.float8e4, kind="ExternalInput"
    ).ap()
    W = nc.dram_tensor(
        "wt", [nt_n, P, kt_n, nb], mybir.dt.float8e4, kind="ExternalInput"
    ).ap()
    Y = nc.dram_tensor(
        "y", [mt_n * P, nt_n * nb], mybir.dt.float16, kind="ExternalOutput"
    ).ap()
    with tile.TileContext(nc) as tc:
        _emit(nc, tc, mybir, X, W, Y, mt_n, nt_n, kt_n, nb)
    nc.compile()
    if hw:
        nc.m = get_hw_module(nc.m)
    return nc


def _get_nc():
    if "nc" not in _NC_CACHE:
        _NC_CACHE["nc"] = _build()
    return _NC_CACHE["nc"]


def _quantize(a):
    # OCP e4m3fn RNE cast (matches jax astype), then reinterpret as the
    # IEEE e4m3 dtype the BIR tensor declares (identical bits below 240).
    return a.astype(ml_dtypes.float8_e4m3fn).view(ml_dtypes.float8_e4m3)


def _in_maps(x, W):
    xq = _quantize(np.ascontiguousarray(x))
    wq = _quantize(np.ascontiguousarray(W))
    # wt[nt, p, kt, n] = wq[nt*NB + n, kt*P + p]
    wt = np.ascontiguousarray(wq.reshape(NT, NB, KT, P).transpose(0, 3, 2, 1))
    maps = []
    for c in range(N_CORES):
        xc = xq[c * MC : (c + 1) * MC]
        # xt[mt, p, kt, m] = xc[mt*P + m, kt*P + p]
        xt = np.ascontiguousarray(xc.reshape(MT, P, KT, P).transpose(0, 3, 2, 1))
        maps.append({"xt": xt, "wt": wt})
    return maps


def _ensure_axon_ntff_hook():
    # Under axon, run_bass_kernel_spmd(trace=True) imports
    # antenv.axon_hooks, which some images lack even though the boot
    # machinery that implements the hook is present.  Register a shim so
    # tracing degrades gracefully instead of raising.
    import sys

    if "antenv.axon_hooks" in sys.modules:
        return
    try:
        from concourse._compat import axon_active

        if not axon_active():
            return
        import importlib.util

        if importlib.util.find_spec("antenv.axon_hooks") is not None:
            return
        import types

        import antenv

        hook = None
        try:
            import trn_agent_boot.trn_boot as _tb

            hook = _tb._ntff_profile_via_ctypes("/opt/axon/libaxon_pjrt.so")
        except Exception:
            hook = None
        mod = types.ModuleType("antenv.axon_hooks")
        mod._hook = hook
        mod.get_axon_ntff_profile_hook = lambda: mod._hook
        def _set(h):
            mod._hook = h
        mod.set_axon_ntff_profile_hook = _set
        antenv.axon_hooks = mod
        sys.modules["antenv.axon_hooks"] = mod
    except Exception:
        pass


def _run(in_maps, trace=False):
    from concourse.bass_utils import run_bass_kernel_spmd

    _ensure_axon_ntff_hook()
    nc = _get_nc()
    return run_bass_kernel_spmd(
        nc, in_maps, core_ids=list(range(len(in_maps))), trace=trace
    )


def kernel(x, W):
    res = _run(_in_maps(x, W))
    return np.concatenate(
        [res.results[c]["y"] for c in range(N_CORES)], axis=0
    ).astype(np.float32, copy=False)
